# revision 1
# baseline (speedup 1.0000x reference)
"""Trainium2 Bass kernel for the additive-attention layer.

Math (per batch b):
    pre[s, h]   = enc[b] @ W2 + hidden[b] @ W1 + b_attn      (W1=W_attn[:H], W2=W_attn[H:])
    energy      = tanh(pre)
    scores[s]   = energy @ w_v (+ b_v, irrelevant: softmax is shift-invariant)
    attn        = softmax(scores)
    context     = attn @ enc[b]

Distribution: data-parallel over batch, 4 batches per core, no collectives.

Device dataflow per (batch, s-chunk of 512):
  - encT tiles (d on partitions, s free) arrive via DMA (host pre-transposes
    + casts to bf16; d-major layout is required because the tensor engine
    contracts over the partition dim).
  - TensorE: psum[h128, s512] += W2[d128, h128].T @ encT[d128, s512]  (16 d-tiles)
  - ScalarE: energy = tanh(psum + hproj[b]) -> SBUF bf16   (hproj = W1.T@hidden + b_attn,
    added as the per-partition activation bias; h lives on partitions)
  - TensorE: scores_psum[1, s512] += w_v[h128, 1].T @ energy  (8 h-tiles)
  - ScalarE: p = exp(scores) -> attn row, accum_out = chunk denominator
    (no max subtraction: scores are O(1) for this data regime, exp is safe)
  - GpSimd: broadcast p across 128 partitions
  - VectorE: ctx_partial[d128, 1] = sum_s encT[d128, s] * p[s]  (16 d-tiles,
    tensor_tensor_reduce; partials per chunk, summed at the end)
Final: denominators -> reciprocal -> scale attn rows and context partials.
"""

import numpy as np
import ml_dtypes
from contextlib import ExitStack

import concourse.bacc as bacc
import concourse.bass as bass
import concourse.tile as tile
import concourse.mybir as mybir
from concourse.bass_utils import run_bass_kernel_spmd

B, S, H = 32, 2048, 1024
D = 2 * H                     # encoder feature dim
NCORES = 8
BPC = B // NCORES             # batches per core
SCH = 512                     # s-chunk (one PSUM bank of fp32)
NCH = S // SCH
NDT = D // 128                # d-tiles (contraction tiles for main matmul)
NHT = H // 128                # h-tiles
NKT = H // 128                # d-tiles for the W1 projection

BF16 = mybir.dt.bfloat16
F32 = mybir.dt.float32
F32R = mybir.dt.float32r

_CACHE = {}


def _build(reps=1, bench_mode=False, encp_bufs=3, enp_bufs=12, ppre_bufs=4,
           psc_bufs=2, scr_bufs=3, ablate="", ctx_mode="gp", dve_split=10):
    # ablate: comma-set of {"noctx", "noscores"} for bench ablations
    # ctx_mode: "gp" = gpsimd broadcast + ACT reduce;
    #           "pe" = PE ones-matmul broadcast + split DVE/ACT reduces
    # dve_split: in "pe" mode, how many of the 16 d-tile reduces go to DVE
    nc = bacc.Bacc("TRN2", target_bir_lowering=False, debug=False)

    # bench_mode: big inputs become device-resident Internal tensors
    # (garbage data) so repeated timed executions don't ship 300MB through
    # the axon tunnel; engine timing is data-independent.
    kind = "Internal" if bench_mode else "ExternalInput"
    encT = nc.dram_tensor("enct", (BPC, D, S), BF16, kind=kind).ap()
    w2 = nc.dram_tensor("w2", (D, H), BF16, kind=kind).ap()
    w1 = nc.dram_tensor("w1", (H, H), BF16, kind=kind).ap()
    hidT = nc.dram_tensor("hidt", (H, BPC), BF16, kind=kind).ap()
    wv = nc.dram_tensor("wv", (H,), BF16, kind=kind).ap()
    ba = nc.dram_tensor("ba", (H,), F32, kind=kind).ap()
    ctx_out = nc.dram_tensor("ctx", (BPC, D), F32, kind="ExternalOutput").ap()
    attn_out = nc.dram_tensor("attn", (BPC, S), F32, kind="ExternalOutput").ap()

    with tile.TileContext(nc) as tc, ExitStack() as ctx:
        weights = ctx.enter_context(tc.tile_pool(name="weights", bufs=1))
        encp = ctx.enter_context(tc.tile_pool(name="encp", bufs=encp_bufs))
        enp = ctx.enter_context(tc.tile_pool(name="enp", bufs=enp_bufs))
        small = ctx.enter_context(tc.tile_pool(name="small", bufs=1))
        bcp = ctx.enter_context(tc.tile_pool(name="bcp", bufs=2))
        scr = ctx.enter_context(tc.tile_pool(name="scr", bufs=scr_bufs))
        ppre = ctx.enter_context(tc.tile_pool(name="ppre", bufs=ppre_bufs, space="PSUM"))
        psc = ctx.enter_context(tc.tile_pool(name="psc", bufs=psc_bufs, space="PSUM"))
        pmisc = ctx.enter_context(tc.tile_pool(name="pmisc", bufs=2, space="PSUM"))

        # --- resident weights ---
        w2_sb = weights.tile([128, NDT, H], BF16)
        nc.sync.dma_start(out=w2_sb, in_=w2.rearrange("(k p) h -> p k h", p=128))
        w1_sb = weights.tile([128, NKT, H], BF16)
        nc.sync.dma_start(out=w1_sb, in_=w1.rearrange("(k p) h -> p k h", p=128))
        hidT_sb = small.tile([128, NKT, BPC], BF16)
        nc.sync.dma_start(out=hidT_sb, in_=hidT.rearrange("(k p) b -> p k b", p=128))
        wv_sb = small.tile([128, NHT], BF16)
        nc.sync.dma_start(out=wv_sb, in_=wv.rearrange("(j p) -> p j", p=128))
        ba_sb = small.tile([128, NHT], F32)
        nc.sync.dma_start(out=ba_sb, in_=ba.rearrange("(j p) -> p j", p=128))
        ones_sb = small.tile([1, 128], F32)
        nc.vector.memset(ones_sb, 1.0)

        for _rep in range(reps):
            # --- persistent accumulators ---
            # (engines can't address partition offsets 1..3, so per-batch rows
            # live as separate partition-0 tiles)
            hproj = small.tile([128, NHT, BPC], F32, name="hproj", tag="hproj")
            attn_rows = [
                small.tile([1, S], F32, name=f"attnrow{b}", tag=f"attnrow{b}")
                for b in range(BPC)
            ]
            denp = small.tile([1, BPC * NCH], F32, name="denp", tag="denp")
            # context partials: column layout (b, k, c)
            ctxp = small.tile([128, BPC * NDT * NCH], F32, name="ctxp", tag="ctxp")
            if ablate:
                nc.vector.memset(ctxp, 0.0)
                nc.vector.memset(denp, 1.0)
                for b in range(BPC):
                    nc.vector.memset(attn_rows[b], 0.0)

            # --- hproj[h, b] = W1.T @ hidden.T + b_attn ---
            for j in range(NHT):
                ph = pmisc.tile([128, BPC], F32)
                for k in range(NKT):
                    nc.tensor.matmul(
                        ph,
                        w1_sb[:, k, j * 128:(j + 1) * 128],
                        hidT_sb[:, k, :],
                        start=(k == 0),
                        stop=(k == NKT - 1),
                    )
                nc.scalar.activation(
                    out=hproj[:, j, :],
                    in_=ph,
                    func=mybir.ActivationFunctionType.Identity,
                    bias=ba_sb[:, j:j + 1],
                    scale=1.0,
                )

            # --- main loop ---
            for b in range(BPC):
                for c in range(NCH):
                    et = encp.tile([128, NDT, SCH], BF16)
                    nc.sync.dma_start(
                        out=et,
                        in_=encT[b, :, c * SCH:(c + 1) * SCH].rearrange(
                            "(k p) s -> p k s", p=128
                        ),
                    )
                    energies = []
                    for j in range(NHT):
                        pp = ppre.tile([128, SCH], F32)
                        for k in range(NDT):
                            nc.tensor.matmul(
                                pp,
                                w2_sb[:, k, j * 128:(j + 1) * 128],
                                et[:, k, :],
                                start=(k == 0),
                                stop=(k == NDT - 1),
                            )
                        en = enp.tile([128, SCH], BF16)
                        nc.scalar.activation(
                            out=en,
                            in_=pp,
                            func=mybir.ActivationFunctionType.Tanh,
                            bias=hproj[:, j, b:b + 1],
                            scale=1.0,
                        )
                        energies.append(en)

                    if "noscores" in ablate:
                        continue
                    ps = psc.tile([1, SCH], F32)
                    for j in range(NHT):
                        nc.tensor.matmul(
                            ps,
                            wv_sb[:, j:j + 1],
                            energies[j],
                            start=(j == 0),
                            stop=(j == NHT - 1),
                        )

                    prow = attn_rows[b][0:1, c * SCH:(c + 1) * SCH]
                    dcol = b * NCH + c
                    nc.scalar.activation(
                        out=prow,
                        in_=ps,
                        func=mybir.ActivationFunctionType.Exp,
                        accum_out=denp[0:1, dcol:dcol + 1],
                    )

                    if "noctx" in ablate:
                        continue
                    # context partials: prod = encT * p (DVE bf16 2x), then a
                    # free-dim reduce (TTR is broken on this HW path, so the
                    # reduce goes to DVE tensor_reduce / ACT Identity accum)
                    pbc = bcp.tile([128, SCH], BF16)
                    if ctx_mode == "gp":
                        prow_bf = bcp.tile([1, SCH], BF16)
                        nc.vector.tensor_copy(prow_bf, prow)
                        nc.gpsimd.partition_broadcast(pbc, prow_bf)
                    else:
                        # broadcast p across partitions on the tensor engine:
                        # ones(1,128).T @ p(1,512), f32r full-rate
                        psbc = pmisc.tile([128, SCH], F32, name="psbc", tag="ph")
                        nc.tensor.matmul(
                            psbc,
                            ones_sb.bitcast(F32R),
                            prow.bitcast(F32R),
                            start=True,
                            stop=True,
                        )
                        nc.vector.tensor_copy(pbc, psbc)

                    for k in range(NDT):
                        col = (b * NDT + k) * NCH + c
                        prod = scr.tile([128, SCH], BF16)
                        nc.vector.tensor_mul(prod, et[:, k, :], pbc)
                        if ctx_mode != "gp" and k < dve_split:
                            nc.vector.tensor_reduce(
                                ctxp[:, col:col + 1],
                                prod,
                                axis=mybir.AxisListType.X,
                                op=mybir.AluOpType.add,
                            )
                        else:
                            prod2 = scr.tile([128, SCH], BF16)
                            nc.scalar.activation(
                                out=prod2,
                                in_=prod,
                                func=mybir.ActivationFunctionType.Identity,
                                accum_out=ctxp[:, col:col + 1],
                            )

            # --- finalize ---
            # per-batch denominators: reduce (1, b, c) over c -> (1, b)
            dent = small.tile([1, BPC], F32)
            nc.vector.tensor_reduce(
                dent,
                denp.rearrange("p (b c) -> p b c", c=NCH),
                axis=mybir.AxisListType.X,
                op=mybir.AluOpType.add,
            )
            rv = small.tile([1, BPC], F32)
            nc.vector.reciprocal(rv, dent)

            # reduce context chunk partials: (128, b*k, c) -> (128, b*k)
            ctxr = small.tile([128, BPC * NDT], F32)
            nc.vector.tensor_reduce(
                ctxr,
                ctxp.rearrange("p (x c) -> p x c", c=NCH),
                axis=mybir.AxisListType.X,
                op=mybir.AluOpType.add,
            )
            for b in range(BPC):
                attn_f = bcp.tile([1, S], F32)
                nc.vector.tensor_scalar_mul(attn_f, attn_rows[b], rv[0:1, b:b + 1])
                nc.sync.dma_start(out=attn_out[b:b + 1, :], in_=attn_f)

                rvb = bcp.tile([128, 1], F32)
                nc.gpsimd.partition_broadcast(rvb, rv[0:1, b:b + 1])
                ctxf = bcp.tile([128, NDT], F32)
                nc.vector.tensor_scalar_mul(
                    ctxf, ctxr[:, b * NDT:(b + 1) * NDT], rvb
                )
                nc.sync.dma_start(
                    out=ctx_out[b].rearrange("(k p) -> p k", p=128),
                    in_=ctxf,
                )

    nc.compile()
    return nc


def _get_nc():
    if "nc" not in _CACHE:
        _CACHE["nc"] = _build()
    return _CACHE["nc"]


def _prep_inputs(hidden, encoder_outputs, W_attn, b_attn, w_v, b_v):
    bf16 = ml_dtypes.bfloat16
    w1 = np.ascontiguousarray(W_attn[:H]).astype(bf16)
    w2 = np.ascontiguousarray(W_attn[H:]).astype(bf16)
    wv_ = w_v.astype(bf16)
    ba_ = np.asarray(b_attn, dtype=np.float32)
    enc_bf = encoder_outputs.astype(bf16)  # cast first (fast), transpose per core
    in_maps = []
    for core in range(NCORES):
        sl = slice(core * BPC, (core + 1) * BPC)
        encT = np.ascontiguousarray(np.swapaxes(enc_bf[sl], 1, 2))
        hidT = np.ascontiguousarray(hidden[sl].T).astype(bf16)
        in_maps.append(
            {
                "enct": encT,
                "w2": w2,
                "w1": w1,
                "hidt": hidT,
                "wv": wv_,
                "ba": ba_,
            }
        )
    return in_maps


def kernel(hidden, encoder_outputs, W_attn, b_attn, w_v, b_v, _trace=False):
    nc = _get_nc()
    in_maps = _prep_inputs(hidden, encoder_outputs, W_attn, b_attn, w_v, b_v)
    res = run_bass_kernel_spmd(
        nc, in_maps, core_ids=list(range(NCORES)), trace=_trace
    )
    context = np.concatenate([r["ctx"] for r in res.results], axis=0)
    attn = np.concatenate([r["attn"] for r in res.results], axis=0)
    if _trace:
        _CACHE["last_results"] = res
    return context, attn

